# revision 14
# baseline (speedup 1.0000x reference)
"""Trainium2 Bass kernel for ConcatAttention.

Reference computation (per batch b):
    q_h = Q @ W1[:, :256].T            # [SQ, 512]
    k_h = K @ W1[:, 256:].T            # [SK, 512]
    hidden[q,k,h] = q_h[q,h] + k_h[k,h] + b1[h]
    act = leaky_relu(hidden, 0.01)
    scores[q,k] = act @ W2  ;  masked ;  softmax over k ;  @ V

Decomposition used on device (exact):
    leaky_relu(z) = 0.99*relu(z) + 0.01*z
    scores[q,k] = sum_h 0.99*W2[h]*relu(z[q,k,h])        (PE matvec over h-blocks)
                + 0.01*sum_h W2[h]*(k_h[k,h]+b1[h])      (rank-1 over k, + mask)
                + 0.01*sum_h W2[h]*q_h[q,h]              (per-row, folded into exp bias)

Sharding: data-parallel; core c -> batch c//2, query half c%2 (128 q rows).
Per-core layout: h on partitions (4 blocks of 128), k on free dim.
relu(k_hb[h,k] + q_h[q,h]) is one dual-op tensor_scalar (DVE) or one
activation(Relu, bias) (ACT) per (q, h-block) tile; PE reduces over h with
0.99*W2 as stationary weights into a [128q, 256k] PSUM scores tile.
"""

import sys

for _p in ("/opt/trn_rl_repo",):
    if _p not in sys.path:
        sys.path.insert(0, _p)

import numpy as np
import ml_dtypes

import concourse.bass as bass
import concourse.bacc as bacc
import concourse.mybir as mybir
from concourse.tile import TileContext
from concourse.bass_utils import run_bass_kernel_spmd

F32 = mybir.dt.float32
BF16 = mybir.dt.bfloat16

B, SQ, SK, DQ, DK, DV = 4, 256, 256, 256, 256, 256
D = DQ + DK  # 512
P = 128
N_CORES = 8
Q_PER_CORE = SQ // 2  # 128
HB = D // P  # 4 h-blocks
DB = DQ // P  # 2 d-blocks
KB = SK // P  # 2 k-blocks

# producer split over tile index t%16: DVE gets the rest
ACT_SLOTS = frozenset({5, 10, 15})
POOL_SLOTS = frozenset({3, 11})

_PROGRAM = None


def _build_program():
    nc = bacc.Bacc("TRN2", target_bir_lowering=False, debug=False)

    pq = nc.dram_tensor("pq", [DB, P, D + Q_PER_CORE], F32, kind="ExternalInput").ap()
    pk = nc.dram_tensor("pk", [DB, P, D + SK], F32, kind="ExternalInput").ap()
    v = nc.dram_tensor("v", [KB, P, DV], F32, kind="ExternalInput").ap()
    b1d = nc.dram_tensor("b1d", [HB, P, 1], F32, kind="ExternalInput").ap()
    w2b = nc.dram_tensor("w2b", [HB, P, 1], BF16, kind="ExternalInput").ap()
    w2f = nc.dram_tensor("w2f", [HB, P, 1], F32, kind="ExternalInput").ap()
    mneg = nc.dram_tensor("mneg", [1, SK], F32, kind="ExternalInput").ap()
    idt = nc.dram_tensor("idt", [P, P], F32, kind="ExternalInput").ap()
    out = nc.dram_tensor("out", [Q_PER_CORE, DV], F32, kind="ExternalOutput").ap()

    with TileContext(nc) as tc:
        with (
            tc.tile_pool(name="consts", bufs=1) as consts,
            tc.tile_pool(name="proj", bufs=1) as proj,
            tc.tile_pool(name="apool", bufs=6) as apool,
            tc.tile_pool(name="aapool", bufs=4) as aapool,
            tc.tile_pool(name="agpool", bufs=4) as agpool,
            tc.tile_pool(name="spool", bufs=1, space="PSUM") as spool,
            tc.tile_pool(name="ppool", bufs=3, space="PSUM") as ppool,
            tc.tile_pool(name="tail", bufs=2) as tail,
        ):
            # ---- load constants / inputs ----
            pq_sb = [consts.tile([P, D + Q_PER_CORE], F32, tag=f"pq{i}", name=f"pq{i}") for i in range(DB)]
            pk_sb = [consts.tile([P, D + SK], F32, tag=f"pk{i}", name=f"pk{i}") for i in range(DB)]
            v_raw = [consts.tile([P, DV], F32, tag=f"vr{i}", name=f"vr{i}") for i in range(KB)]
            v_sb = [consts.tile([P, DV], F32, tag=f"v{i}", name=f"v{i}") for i in range(KB)]
            b1_raw = [consts.tile([P, 1], F32, tag=f"b1r{i}", name=f"b1r{i}") for i in range(HB)]
            b1_sb = [consts.tile([P, 1], F32, tag=f"b1{i}", name=f"b1{i}") for i in range(HB)]
            w2b_raw = [consts.tile([P, 1], BF16, tag=f"w2br{i}", name=f"w2br{i}") for i in range(HB)]
            w2b_sb = [consts.tile([P, 1], BF16, tag=f"w2b{i}", name=f"w2b{i}") for i in range(HB)]
            w2f_raw = [consts.tile([P, 1], F32, tag=f"w2fr{i}", name=f"w2fr{i}") for i in range(HB)]
            w2f_sb = [consts.tile([P, 1], F32, tag=f"w2f{i}", name=f"w2f{i}") for i in range(HB)]
            mneg_sb = consts.tile([1, SK], F32, tag="mneg", name="mneg")
            ones_sb = consts.tile([1, P], F32, tag="ones", name="ones")
            idt_raw = consts.tile([P, P], F32, tag="idtr", name="idtr")
            idt_sb = consts.tile([P, P], F32, tag="idt", name="idt")

            # chunked loads: [qt|kt] part first, then per-hb weight columns,
            # so hb=0 projections can start after ~1/5 of the bytes land
            for i in range(DB):
                nc.sync.dma_start(out=pq_sb[i][:, D:], in_=pq[i][:, D:])
                nc.sync.dma_start(out=pk_sb[i][:, D:], in_=pk[i][:, D:])
            for hb_ in range(HB):
                cs = slice(hb_ * P, (hb_ + 1) * P)
                for i in range(DB):
                    nc.sync.dma_start(out=pq_sb[i][:, cs], in_=pq[i][:, cs])
                    nc.sync.dma_start(out=pk_sb[i][:, cs], in_=pk[i][:, cs])
            for i in range(KB):
                nc.sync.dma_start(out=v_raw[i], in_=v[i])
            for i in range(HB):
                nc.sync.dma_start(out=b1_raw[i], in_=b1d[i])
                nc.sync.dma_start(out=w2b_raw[i], in_=w2b[i])
                nc.sync.dma_start(out=w2f_raw[i], in_=w2f[i])
            nc.sync.dma_start(out=mneg_sb, in_=mneg)
            nc.sync.dma_start(out=idt_raw, in_=idt)
            # stage PE-consumed small tiles through DVE so PE matmuls wait on
            # at most one new semaphore (walrus: 1 sync wait per LDWEIGHTS)
            for i in range(KB):
                nc.vector.tensor_copy(out=v_sb[i], in_=v_raw[i])
            for i in range(HB):
                nc.vector.tensor_copy(out=w2b_sb[i], in_=w2b_raw[i])
                nc.vector.tensor_copy(out=w2f_sb[i], in_=w2f_raw[i])
                nc.vector.tensor_copy(out=b1_sb[i], in_=b1_raw[i])
            nc.vector.memset(ones_sb, 1.0)
            nc.vector.tensor_copy(out=idt_sb, in_=idt_raw)

            # ---- projections: q_hT[h, q], k_hb[h, k] (h on partitions) ----
            q_hT = [proj.tile([P, Q_PER_CORE], F32, tag=f"qhT{i}", name=f"qhT{i}") for i in range(HB)]
            k_hb = [proj.tile([P, SK], BF16, tag=f"khb{i}", name=f"khb{i}") for i in range(HB)]
            for hb in range(HB):
                hs = slice(hb * P, (hb + 1) * P)
                ps_q = ppool.tile([P, Q_PER_CORE], F32, tag="ps", name="ps_q")
                for db in range(DB):
                    nc.tensor.matmul(
                        ps_q, lhsT=pq_sb[db][:, hs], rhs=pq_sb[db][:, D:],
                        start=(db == 0), stop=(db == DB - 1),
                    )
                nc.vector.tensor_copy(out=q_hT[hb], in_=ps_q)
                ps_k = ppool.tile([P, SK], F32, tag="ps", name="ps_k")
                for db in range(DB):
                    nc.tensor.matmul(
                        ps_k, lhsT=pk_sb[db][:, hs], rhs=pk_sb[db][:, D:],
                        start=(db == 0), stop=(db == DB - 1),
                    )
                # add b1 (per-partition) and cast to bf16
                nc.vector.tensor_scalar(
                    out=k_hb[hb], in0=ps_k, scalar1=b1_sb[hb], scalar2=None,
                    op0=mybir.AluOpType.add,
                )

            # ---- linear-term vectors ----
            # cq[q] = 0.01 * sum_h W2[h] q_h[q, h]   -> [128q, 1]
            ps_cq = ppool.tile([P, 1], F32, tag="ps", name="ps_cq")
            for hb in range(HB):
                nc.tensor.matmul(
                    ps_cq, lhsT=q_hT[hb], rhs=w2f_sb[hb],
                    start=(hb == 0), stop=(hb == HB - 1),
                )
            cq_s = tail.tile([P, 1], F32, tag="cq", name="cq")
            nc.scalar.mul(cq_s, ps_cq, 0.01)

            # ck[k] = 0.01 * sum_h W2[h] (k_h[k,h]+b1[h]) + mneg[k]  -> [1, 256]
            ps_ck = ppool.tile([1, SK], F32, tag="ps", name="ps_ck")
            for hb in range(HB):
                nc.tensor.matmul(
                    ps_ck, lhsT=w2b_sb[hb], rhs=k_hb[hb],
                    start=(hb == 0), stop=(hb == HB - 1),
                )
            ck_t = tail.tile([1, SK], F32, tag="ckt", name="ckt")
            ck_m = tail.tile([1, SK], F32, tag="ck", name="ck")
            # w2b = 0.99*W2 in bf16 -> rescale by 0.01/0.99
            nc.vector.tensor_scalar(
                out=ck_t, in0=ps_ck, scalar1=0.01 / 0.99, scalar2=None,
                op0=mybir.AluOpType.mult,
            )
            nc.vector.tensor_add(out=ck_m, in0=ck_t, in1=mneg_sb)

            # ---- main: scoresT[k, q] (transposed; PSUM matmul outs must
            # start at partition 0, so M=128 with a-tile as stationary).
            # q outer / hb inner: each q-column fully accumulates before the
            # next column's start=True re-marks the bank's zero region, and
            # PSUM reads ignore the pending-zero mark, so no init is needed.
            scoresT_ps = [
                spool.tile([P, Q_PER_CORE], F32, tag=f"scT{i}", name=f"scT{i}")
                for i in range(KB)
            ]
            for q in range(Q_PER_CORE):
                for hb in range(HB):
                    t = q * HB + hb
                    r = t % 16
                    qcol = q_hT[hb][:, q : q + 1]
                    if r in ACT_SLOTS:
                        a = aapool.tile([P, SK], BF16, tag="aa", name="aa")
                        nc.scalar.activation(
                            a, k_hb[hb], mybir.ActivationFunctionType.Relu,
                            bias=qcol,
                        )
                    elif r in POOL_SLOTS:
                        a = agpool.tile([P, SK], BF16, tag="ag", name="ag")
                        nc.gpsimd.tensor_scalar(
                            out=a, in0=k_hb[hb], scalar1=qcol, scalar2=0.0,
                            op0=mybir.AluOpType.add, op1=mybir.AluOpType.max,
                        )
                    else:
                        a = apool.tile([P, SK], BF16, tag="a", name="a")
                        nc.vector.tensor_scalar(
                            out=a, in0=k_hb[hb], scalar1=qcol, scalar2=0.0,
                            op0=mybir.AluOpType.add, op1=mybir.AluOpType.max,
                        )
                    for kb in range(KB):
                        nc.tensor.matmul(
                            scoresT_ps[kb][:, q : q + 1],
                            lhsT=a[:, kb * P : (kb + 1) * P], rhs=w2b_sb[hb],
                            start=(hb == 0), stop=(hb == HB - 1),
                            skip_group_check=True,
                        )

            # transpose back: scores[q, k] in PSUM, then add rank-1 ck term
            scoresT_sb = [
                tail.tile([P, Q_PER_CORE], F32, tag=f"scTs{i}", name=f"scTs{i}")
                for i in range(KB)
            ]
            for kb in range(KB):
                nc.vector.tensor_copy(out=scoresT_sb[kb], in_=scoresT_ps[kb])
            scores = spool.tile([P, SK], F32, tag="scores", name="scores")
            for kb in range(KB):
                nc.tensor.matmul(
                    scores[:, kb * P : (kb + 1) * P], lhsT=scoresT_sb[kb],
                    rhs=idt_sb, is_transpose=True, start=(kb == 0),
                    stop=False, skip_group_check=True,
                )
            nc.tensor.matmul(
                scores, lhsT=ones_sb, rhs=ck_m, start=False, stop=True,
                skip_group_check=True,
            )

            # ---- softmax over k ----
            negmax = tail.tile([P, 1], F32, tag="negmax", name="negmax")
            nc.vector.tensor_reduce(
                negmax, scores, axis=mybir.AxisListType.X, op=mybir.AluOpType.max,
                negate=True,
            )
            ebias = tail.tile([P, 1], F32, tag="ebias", name="ebias")
            nc.scalar.activation(
                ebias, negmax, mybir.ActivationFunctionType.Identity,
                bias=cq_s,
            )
            attn = tail.tile([P, SK], F32, tag="attn", name="attn")
            rowsum = tail.tile([P, 1], F32, tag="rowsum", name="rowsum")
            nc.scalar.activation(
                attn, scores, mybir.ActivationFunctionType.Exp,
                bias=ebias, accum_out=rowsum,
            )
            rcp = tail.tile([P, 1], F32, tag="rcp", name="rcp")
            nc.vector.reciprocal(rcp, rowsum)

            # ---- attn @ V ----
            attnT = [tail.tile([P, P], F32, tag=f"attnT{i}", name=f"attnT{i}") for i in range(KB)]
            for kb in range(KB):
                ps_t = ppool.tile([P, P], F32, tag="ps", name="ps_t")
                nc.tensor.transpose(ps_t, attn[:, kb * P : (kb + 1) * P], idt_sb)
                nc.vector.tensor_copy(out=attnT[kb], in_=ps_t)
            ps_o = ppool.tile([P, DV], F32, tag="ps", name="ps_o")
            for kb in range(KB):
                nc.tensor.matmul(
                    ps_o, lhsT=attnT[kb], rhs=v_sb[kb],
                    start=(kb == 0), stop=(kb == KB - 1),
                )
            out_sb = tail.tile([P, DV], F32, tag="out", name="out")
            nc.vector.tensor_scalar(
                out=out_sb, in0=ps_o, scalar1=rcp, scalar2=None,
                op0=mybir.AluOpType.mult,
            )
            nc.sync.dma_start(out=out, in_=out_sb)

    nc.compile()
    return nc


def _get_program():
    global _PROGRAM
    if _PROGRAM is None:
        _PROGRAM = _build_program()
    return _PROGRAM


def make_in_maps(Q, K, V, mask_out, W1, b1, W2):
    Q = np.asarray(Q, dtype=np.float32)
    K = np.asarray(K, dtype=np.float32)
    V = np.asarray(V, dtype=np.float32)
    W1 = np.asarray(W1, dtype=np.float32)
    b1 = np.asarray(b1, dtype=np.float32)
    W2 = np.asarray(W2, dtype=np.float32)
    mask = np.asarray(mask_out).astype(bool).reshape(B, SK)

    # host-side weight repacking (shared across cores)
    w1qt = np.ascontiguousarray(W1[:, :DQ].T).reshape(DB, P, D)  # [d, h] d-blocks
    w1kt = np.ascontiguousarray(W1[:, DQ:].T).reshape(DB, P, D)
    b1d = b1.reshape(HB, P, 1)
    w2b = (0.99 * W2).reshape(HB, P, 1).astype(ml_dtypes.bfloat16)
    w2f = W2.reshape(HB, P, 1)

    in_maps = []
    for c in range(N_CORES):
        b = c // 2
        q0 = (c % 2) * Q_PER_CORE
        qt = np.ascontiguousarray(Q[b, q0 : q0 + Q_PER_CORE, :].T).reshape(
            DB, P, Q_PER_CORE
        )
        kt = np.ascontiguousarray(K[b].T).reshape(DB, P, SK)
        vv = np.ascontiguousarray(V[b]).reshape(KB, P, DV)
        pq_h = np.concatenate([w1qt, qt], axis=2)
        pk_h = np.concatenate([w1kt, kt], axis=2)
        mneg = np.where(mask[b], np.float32(-1e9), np.float32(0.0)).reshape(1, SK)
        in_maps.append(
            {
                "pq": np.ascontiguousarray(pq_h).astype(np.float32),
                "pk": np.ascontiguousarray(pk_h).astype(np.float32),
                "v": vv.astype(np.float32),
                "b1d": b1d.astype(np.float32),
                "w2b": w2b,
                "w2f": w2f.astype(np.float32),
                "mneg": mneg.astype(np.float32),
                "idt": np.eye(P, dtype=np.float32),
            }
        )

    return in_maps


def assemble(results):
    out = np.empty((B, SQ, DV), dtype=np.float32)
    for c in range(N_CORES):
        b = c // 2
        q0 = (c % 2) * Q_PER_CORE
        out[b, q0 : q0 + Q_PER_CORE, :] = np.asarray(results[c]["out"])
    return out


def kernel(Q, K, V, mask_out, W1, b1, W2):
    in_maps = make_in_maps(Q, K, V, mask_out, W1, b1, W2)
    nc = _get_program()
    res = run_bass_kernel_spmd(nc, in_maps, core_ids=list(range(N_CORES)))
    return assemble(res.results)
